# revision 4
# baseline (speedup 1.0000x reference)
"""MetaSGD (per-element-lr Adam) update kernel for 8 Trainium2 NeuronCores.

Math per element (faithful to the reference):
    bc1 = 1 - 0.9**step ; bc2 = 1 - 0.99**step
    m_new = 0.9*m + 0.1*g
    v_new = 0.99*v + 0.01*g*g
    denom = sqrt(v_new/bc2) + eps
    out   = w - (lr/bc1) * m_new / denom

Sharding: purely elementwise, so each of the 8 cores takes a contiguous
1/8 slice of every tensor along the leading dim (zero-copy numpy views),
viewed as (128, F) for SBUF partitioning.

Per-core per-tile pipeline (f32, free-dim tile = 2048):
    ScalarE: q = Square(0.1*g)             # 0.01*g^2
    VectorE: v = (0.99*v) + q              # affine_then_add custom op
    ScalarE: v = Sqrt(v * 1/bc2)
    ScalarE: v = v + eps                   # Identity w/ bias
    VectorE: q = 1/v                       # reciprocal_approx_fast (~51 ULP)
    VectorE: m = (m + g/9) * (0.9/bc1)     # ln_bwd_dx custom op = m_new/bc1
    VectorE: m = m * q
    VectorE: m = m * lr
    VectorE: w = w - m
"""

import numpy as np

import concourse.bacc as bacc
import concourse.mybir as mybir
from concourse.bass_utils import run_bass_kernel_spmd
from concourse.tile import TileContext

BETA1 = 0.9
BETA2 = 0.99
EPS = 1e-8

P = 128  # SBUF partitions
N_CORES = 8
TILE = 2048  # free-dim elements per tile (1 MiB per 128xTILE f32 tile)

# (suffix, rows-per-core in the full tensor, full row width)
_GROUPS = [
    ("a", 256, 8192),   # w0 etc: (2048, 8192) -> per core (256, 8192) = (128, 16384)
    ("b", 1024, 2048),  # w1 etc: (8192, 2048) -> per core (1024, 2048) = (128, 16384)
    ("c", 1024, 1),     # w2 etc: (8192,)      -> per core (1024,)      = (128, 8)
]
_NAMES = ("w", "g", "lr", "m", "v")

_CACHE: dict[int, object] = {}


def _build(step: int):
    bc1 = 1.0 - BETA1**step
    bc2 = 1.0 - BETA2**step
    AF = mybir.ActivationFunctionType
    OP = mybir.AluOpType
    f32 = mybir.dt.float32

    nc = bacc.Bacc("TRN2", target_bir_lowering=False, debug=False)

    dram_in = {}
    dram_out = {}
    regions = []  # (suffix, free_len)
    for suf, rows, cols in _GROUPS:
        free = rows * cols // P
        regions.append((suf, free))
        for nm in _NAMES:
            dram_in[nm + suf] = nc.dram_tensor(
                nm + suf, [P, free], f32, kind="ExternalInput"
            )
        dram_out[suf] = nc.dram_tensor("o" + suf, [P, free], f32, kind="ExternalOutput")

    with TileContext(nc) as tc:
        with (
            tc.tile_pool(name="const", bufs=1) as cpool,
            tc.tile_pool(name="pool", bufs=3) as pool,
        ):
            eps_t = cpool.tile([P, 1], f32, name="eps_t")
            nc.vector.memset(eps_t[:], EPS)
            for suf, free in regions:
                for i in range(0, free, TILE):
                    fw = min(TILE, free - i)
                    sl = slice(i, i + fw)

                    tw = pool.tile([P, TILE], f32, name="tw", tag="w")
                    tg = pool.tile([P, TILE], f32, name="tg", tag="g")
                    tl = pool.tile([P, TILE], f32, name="tl", tag="lr")
                    tm = pool.tile([P, TILE], f32, name="tm", tag="m")
                    tv = pool.tile([P, TILE], f32, name="tv", tag="v")
                    tq = pool.tile([P, TILE], f32, name="tq", tag="q")

                    nc.sync.dma_start(out=tw[:, :fw], in_=dram_in["w" + suf][:, sl])
                    nc.sync.dma_start(out=tg[:, :fw], in_=dram_in["g" + suf][:, sl])
                    nc.sync.dma_start(out=tl[:, :fw], in_=dram_in["lr" + suf][:, sl])
                    nc.sync.dma_start(out=tm[:, :fw], in_=dram_in["m" + suf][:, sl])
                    nc.sync.dma_start(out=tv[:, :fw], in_=dram_in["v" + suf][:, sl])

                    w, g, lr, m, v, q = (
                        t[:, :fw] for t in (tw, tg, tl, tm, tv, tq)
                    )
                    # q = (0.1*g)^2 = 0.01*g^2
                    nc.scalar.activation(q, g, AF.Square, scale=0.1)
                    # v = 0.99*v + q  (= v_new)
                    nc.vector.affine_then_add(v, v, q, scale=BETA2, bias=0.0)
                    # v = sqrt(v_new/bc2)
                    nc.scalar.activation(v, v, AF.Sqrt, scale=1.0 / bc2)
                    # v = v + eps  (= denom)
                    nc.scalar.activation(v, v, AF.Identity, bias=eps_t[:])
                    # q = 1/denom
                    nc.vector.reciprocal_approx_fast(q, v)
                    # m = (m - g*(-1/9) - 0)*(0.9/bc1) = (0.9m + 0.1g)/bc1
                    nc.vector.ln_bwd_dx(m, m, g, -1.0 / 9.0, 0.0, BETA1 / bc1)
                    # m = m_new/bc1 * 1/denom
                    nc.vector.tensor_tensor(m, m, q, OP.mult)
                    # m = full update
                    nc.vector.tensor_tensor(m, m, lr, OP.mult)
                    # w = w - update
                    nc.vector.tensor_tensor(w, w, m, OP.subtract)

                    nc.sync.dma_start(out=dram_out[suf][:, sl], in_=w)
    nc.finalize()
    return nc


def _shard(inputs: dict) -> list[dict]:
    """Slice full inputs into 8 per-core maps of (128, F) views (no copies)."""
    arrs = {k: np.ascontiguousarray(np.asarray(v, dtype=np.float32))
            for k, v in inputs.items() if k != "step"}
    in_maps = []
    for k in range(N_CORES):
        m = {}
        for p, (suf, rows, cols) in enumerate(_GROUPS):
            free = rows * cols // P
            for nm in _NAMES:
                full = arrs[f"{nm}{p}"]
                m[nm + suf] = full.reshape(-1)[
                    k * rows * cols : (k + 1) * rows * cols
                ].reshape(P, free)
        in_maps.append(m)
    return in_maps


def _unshard(results: list[dict]):
    out0 = np.concatenate(
        [results[k]["oa"].reshape(256, 8192) for k in range(N_CORES)], axis=0
    )
    out1 = np.concatenate(
        [results[k]["ob"].reshape(1024, 2048) for k in range(N_CORES)], axis=0
    )
    out2 = np.concatenate(
        [results[k]["oc"].reshape(-1) for k in range(N_CORES)], axis=0
    )
    return out0, out1, out2


def _run(inputs: dict, **kwargs):
    step = int(inputs["step"])
    if step not in _CACHE:
        _CACHE[step] = _build(step)
    nc = _CACHE[step]
    res = run_bass_kernel_spmd(nc, _shard(inputs), core_ids=list(range(N_CORES)),
                               **kwargs)
    return _unshard(res.results), res


def kernel(**inputs):
    outs, _ = _run(inputs)
    return outs


def run_traced(**inputs):
    """Like kernel() but profiles; returns (outputs, BassKernelResults)."""
    return _run(inputs, trace=True)


# revision 7
# speedup vs baseline: 16.1339x; 16.1339x over previous
"""MetaSGD (per-element-lr Adam) update kernel for 8 Trainium2 NeuronCores.

Math per element (faithful to the reference):
    bc1 = 1 - 0.9**step ; bc2 = 1 - 0.99**step
    m_new = 0.9*m + 0.1*g
    v_new = 0.99*v + 0.01*g*g
    denom = sqrt(v_new/bc2) + eps
    out   = w - (lr/bc1) * m_new / denom

Sharding: purely elementwise, so each of the 8 cores takes a contiguous
1/8 slice of every tensor along the leading dim (zero-copy numpy views),
viewed as (128, F) for SBUF partitioning.

Per-core per-tile pipeline (f32, free-dim tile = 2048), engine-balanced so
DVE (4 ops), GpSimd (2 ops), ScalarE (3 ops) all fit under the ~14us/tile
DMA floor:
    ScalarE: q = Square(0.1*g)             # 0.01*g^2
    VectorE: v = (0.99*v) + q              # affine_then_add custom op
    ScalarE: v = Sqrt(v * 1/bc2)
    ScalarE: v = v + eps                   # Identity w/ bias
    VectorE: q = 1/v                       # reciprocal_approx_fast (~51 ULP)
    VectorE: m = (m + g/9) * (0.9/bc1)     # ln_bwd_dx custom op = m_new/bc1
    GpSimd:  m = m * lr
    VectorE: m = m * q
    GpSimd:  w = w - m
The five loads issue on the SP HWDGE ring, the store on the ACT ring.
"""

import numpy as np

import concourse.bacc as bacc
import concourse.mybir as mybir
from concourse.bass_utils import run_bass_kernel_spmd
from concourse.tile import TileContext

BETA1 = 0.9
BETA2 = 0.99
EPS = 1e-8

P = 128  # SBUF partitions
N_CORES = 8
TILE = 2048  # free-dim elements per tile (1 MiB per 128xTILE f32 tile)

# (suffix, rows-per-core in the full tensor, full row width)
_GROUPS = [
    ("a", 256, 8192),   # w0 etc: (2048, 8192) -> per core (256, 8192) = (128, 16384)
    ("b", 1024, 2048),  # w1 etc: (8192, 2048) -> per core (1024, 2048) = (128, 16384)
    ("c", 1024, 1),     # w2 etc: (8192,)      -> per core (1024,)      = (128, 8)
]
_NAMES = ("w", "g", "lr", "m", "v")

_CACHE: dict[int, object] = {}


def _build(step: int):
    bc1 = 1.0 - BETA1**step
    bc2 = 1.0 - BETA2**step
    AF = mybir.ActivationFunctionType
    OP = mybir.AluOpType
    f32 = mybir.dt.float32

    nc = bacc.Bacc("TRN2", target_bir_lowering=False, debug=False)

    dram_in = {}
    dram_out = {}
    regions = []  # (suffix, free_len)
    for suf, rows, cols in _GROUPS:
        free = rows * cols // P
        regions.append((suf, free))
        for nm in _NAMES:
            dram_in[nm + suf] = nc.dram_tensor(
                nm + suf, [P, free], f32, kind="ExternalInput"
            )
        dram_out[suf] = nc.dram_tensor("o" + suf, [P, free], f32, kind="ExternalOutput")

    with TileContext(nc) as tc:
        with (
            tc.tile_pool(name="const", bufs=1) as cpool,
            tc.tile_pool(name="pool", bufs=3) as pool,
        ):
            eps_t = cpool.tile([P, 1], f32, name="eps_t")
            nc.vector.memset(eps_t[:], EPS)
            for suf, free in regions:
                for i in range(0, free, TILE):
                    fw = min(TILE, free - i)
                    sl = slice(i, i + fw)

                    tw = pool.tile([P, TILE], f32, name="tw", tag="w")
                    tg = pool.tile([P, TILE], f32, name="tg", tag="g")
                    tl = pool.tile([P, TILE], f32, name="tl", tag="lr")
                    tm = pool.tile([P, TILE], f32, name="tm", tag="m")
                    tv = pool.tile([P, TILE], f32, name="tv", tag="v")
                    tq = pool.tile([P, TILE], f32, name="tq", tag="q")

                    nc.sync.dma_start(out=tw[:, :fw], in_=dram_in["w" + suf][:, sl])
                    nc.sync.dma_start(out=tg[:, :fw], in_=dram_in["g" + suf][:, sl])
                    nc.sync.dma_start(out=tl[:, :fw], in_=dram_in["lr" + suf][:, sl])
                    nc.sync.dma_start(out=tm[:, :fw], in_=dram_in["m" + suf][:, sl])
                    nc.sync.dma_start(out=tv[:, :fw], in_=dram_in["v" + suf][:, sl])

                    w, g, lr, m, v, q = (
                        t[:, :fw] for t in (tw, tg, tl, tm, tv, tq)
                    )
                    # q = (0.1*g)^2 = 0.01*g^2
                    nc.scalar.activation(q, g, AF.Square, scale=0.1)
                    # v = 0.99*v + q  (= v_new)
                    nc.vector.affine_then_add(v, v, q, scale=BETA2, bias=0.0)
                    # v = sqrt(v_new/bc2)
                    nc.scalar.activation(v, v, AF.Sqrt, scale=1.0 / bc2)
                    # v = v + eps  (= denom)
                    nc.scalar.activation(v, v, AF.Identity, bias=eps_t[:])
                    # q = 1/denom
                    nc.vector.reciprocal_approx_fast(q, v)
                    # m = (m - g*(-1/9) - 0)*(0.9/bc1) = (0.9m + 0.1g)/bc1
                    nc.vector.ln_bwd_dx(m, m, g, -1.0 / 9.0, 0.0, BETA1 / bc1)
                    # m = m * lr  (on GpSimd — offloads the DVE)
                    nc.gpsimd.tensor_tensor(m, m, lr, OP.mult)
                    # m = full update
                    nc.vector.tensor_tensor(m, m, q, OP.mult)
                    # w = w - update  (on GpSimd)
                    nc.gpsimd.tensor_tensor(w, w, m, OP.subtract)

                    # out-DMA on the ACT HWDGE ring: balances the two
                    # hardware descriptor-generation rings (SP carries the
                    # five loads) — measurably faster than all-on-SP.
                    nc.scalar.dma_start(out=dram_out[suf][:, sl], in_=w)
    nc.finalize()
    return nc


def _shard(inputs: dict) -> list[dict]:
    """Slice full inputs into 8 per-core maps of (128, F) views (no copies)."""
    arrs = {k: np.ascontiguousarray(np.asarray(v, dtype=np.float32))
            for k, v in inputs.items() if k != "step"}
    in_maps = []
    for k in range(N_CORES):
        m = {}
        for p, (suf, rows, cols) in enumerate(_GROUPS):
            free = rows * cols // P
            for nm in _NAMES:
                full = arrs[f"{nm}{p}"]
                m[nm + suf] = full.reshape(-1)[
                    k * rows * cols : (k + 1) * rows * cols
                ].reshape(P, free)
        in_maps.append(m)
    return in_maps


def _unshard(results: list[dict]):
    out0 = np.concatenate(
        [results[k]["oa"].reshape(256, 8192) for k in range(N_CORES)], axis=0
    )
    out1 = np.concatenate(
        [results[k]["ob"].reshape(1024, 2048) for k in range(N_CORES)], axis=0
    )
    out2 = np.concatenate(
        [results[k]["oc"].reshape(-1) for k in range(N_CORES)], axis=0
    )
    return out0, out1, out2


def _run(inputs: dict, **kwargs):
    step = int(inputs["step"])
    if step not in _CACHE:
        _CACHE[step] = _build(step)
    nc = _CACHE[step]
    res = run_bass_kernel_spmd(nc, _shard(inputs), core_ids=list(range(N_CORES)),
                               **kwargs)
    return _unshard(res.results), res


def kernel(**inputs):
    outs, _ = _run(inputs)
    return outs


def run_traced(**inputs):
    """Like kernel() but profiles; returns (outputs, BassKernelResults)."""
    return _run(inputs, trace=True)
